# revision 1
# baseline (speedup 1.0000x reference)
"""Trainium2 Bass kernel for nn_CameraEstimator.

Computes, for each batch item b:
    camera[b] = einsum('chw,c->hw', x[b], W)          (C=256 contraction)
    out[b]    = nearest-rotation(camera[b])           (SVD u@vh + det reflection fix)

The SVD-based orthonormalization is replaced by a determinant-scaled Newton
polar iteration plus a closed-form smallest-eigenvalue reflection correction:
    orth = polar(camera)            (Newton: Y <- 0.5*(mu*Y + (mu*det)^-1 * cof(Y)))
    P = orth^T camera = V S V^T;  s3 = smallest eig of P (trig formula + polish)
    proj = adj(P - s3 I)/tr(...) = v3 v3^T
    out = orth - (1 - sign(det)) * orth @ proj

Sharding: batch dim split evenly across 8 NeuronCores (data parallel), W
replicated. All math is done on [128, TPC, 9]-shaped fp32 SBUF planes
(partition = b mod, free = (tile, matrix-entry)).
"""

import os
import numpy as np

import concourse.bacc as bacc
import concourse.bass as bass
import concourse.mybir as mybir
from concourse.bass_types import AP
from concourse.tile import TileContext
from concourse import bass_utils

F32 = mybir.dt.float32
ALU = mybir.AluOpType
ACT = mybir.ActivationFunctionType

B_FULL = 32768
C = 256
E = 9
N_CORES = 8
P = 128
B_LOCAL = B_FULL // N_CORES          # 4096
TPC = B_LOCAL // P                   # 32 matrices per partition

NEWTON_ITERS = 6
SCALED_ITERS = 3
POLISH_ITERS = 2


def v(base: AP, off: int, *dims) -> AP:
    """Free-dim view of an SBUF tile AP: keep partition dim, set free dims.

    dims are (step, count) pairs in element units relative to the tile row.
    """
    return AP(base.tensor, base.offset + off,
              [list(base.ap[0])] + [[s, c] for (s, c) in dims])


def make_wm(W: np.ndarray) -> np.ndarray:
    """Split-fp16 masked-W moving operands for the PE contraction.

    wm[j, k, m] (m < 9)  = fp16(W[c]) where c = (128j+k)//9, if (128j+k)%9 == m
    wm[j, k, 9+m]        = fp16(W[c] - fp16(W[c])) at the same positions,
    so that xT16_j.T @ wm[j] accumulates [camera_hi | camera_lo] in fp32 PSUM
    and camera = hi + lo recovers full fp32 W precision (x is fp16-rounded).
    """
    kidx = np.arange(2304)
    wh = np.zeros((2304, E), np.float32)
    wh[kidx, kidx % E] = W[kidx // E]
    hi = wh.astype(np.float16)
    lo = (wh - hi.astype(np.float32)).astype(np.float16)
    wm = np.concatenate([hi, lo], axis=1)    # [2304, 18] fp16
    return np.ascontiguousarray(wm.reshape(18, P, 18))


def _emit(nc, tc, x_ap, w_ap, wm_ap, y_ap):
    f32 = F32
    vec = nc.vector
    act = nc.scalar
    STAGE = int(os.environ.get("KERNEL_STAGE", "99"))

    # b = p*TPC + t so that the output DMA is one fully-contiguous transfer
    x_flat = x_ap.rearrange("b c h w -> b (c h w)")
    x_tiled = x_flat.rearrange("(p t) f -> p t f", p=P)
    y_flat = y_ap.rearrange("b h w -> b (h w)").rearrange("(p t) e -> p (t e)", p=P)

    F16 = mybir.dt.float16
    NCH = (C * E) // P                       # 18 chunks of 128 per tile
    DVE_SHARE = float(os.environ.get("KERNEL_DVE_SHARE", "0.6"))

    with tc.tile_pool(name="xin", bufs=3) as xpool, \
         tc.tile_pool(name="tp", bufs=6, space="PSUM") as tpp, \
         tc.tile_pool(name="pcp", bufs=2, space="PSUM") as pcp, \
         tc.tile_pool(name="wk", bufs=1) as wp:
        from concourse.masks import make_identity

        idt = wp.tile([P, P], F16)
        make_identity(nc, idt[:])
        # split-fp16 masked W: wm_sb[k, j*18 + m] (m<9: hi, m>=9: lo)
        wm_sb = wp.tile([P, NCH * 18], F16)
        nc.sync.dma_start(
            out=wm_sb[:],
            in_=AP(wm_ap.tensor, 0, [[18, P], [18 * P, NCH], [1, 18]]))

        cam = wp.tile([P, TPC * E], f32)     # camera matrices, compact (t, e)

        # W replicated + expanded for the DVE-path tiles
        DVE_TILES = int(os.environ.get("KERNEL_DVE_TILES", "8"))
        w_row = wp.tile([P, C], f32)
        w_rep = wp.tile([P, C], f32)
        w_big = wp.tile([P, C * E], f32)
        if DVE_TILES:
            nc.sync.dma_start(out=w_row[:1, :],
                              in_=AP(w_ap.tensor, 0, [[1, 1], [1, C]]))
            nc.gpsimd.partition_broadcast(w_rep[:], w_row[:1, :])
            vec.tensor_copy(v(w_big, 0, (E, C), (1, E)),
                            v(w_rep, 0, (1, C), (0, E)))
        dve_set = {round((i + 0.5) * TPC / DVE_TILES) for i in range(DVE_TILES)} \
            if DVE_TILES else set()

        # ---- contraction on the PE in fp16 with split-fp16 W --------------
        # per tile: convert x to fp16, transpose the 18 [128,128] chunks on
        # the PE (fp16 -> fast weight load), copy back to SBUF, then 18
        # accumulating matmuls xT16_j.T @ [wm_hi_j | wm_lo_j] -> [128(b), 18]
        # fp32 in PSUM; camera = hi-half + lo-half.
        job = 0
        for t in range(TPC):
            xt = xpool.tile([P, C * E], f32, tag="xt", name=f"xt{t}")
            nc.sync.dma_start(out=xt[:], in_=x_tiled[:, t, :])
            if t in dve_set:
                # exact fp32 path on DVE: elementwise mult, halving folds,
                # small strided tail reduce
                prod = xpool.tile([P, C * E], f32, tag="prod", name=f"prod{t}")
                vec.tensor_tensor(prod[:], xt[:], w_big[:], ALU.mult)
                n = C * E
                while n > 72:
                    n //= 2
                    vec.tensor_tensor(prod[:, :n], prod[:, :n],
                                      prod[:, n:2 * n], ALU.add)
                vec.tensor_reduce(v(cam, t * E, (1, E)),
                                  v(prod, 0, (1, E), (E, 8)),
                                  mybir.AxisListType.X, ALU.add)
                continue
            xt16 = xpool.tile([P, C * E], F16, tag="xt16", name=f"xt16_{t}")
            job += 1
            if (job * DVE_SHARE) % 1.0 < DVE_SHARE:
                vec.tensor_copy(xt16[:], xt[:])
            else:
                act.copy(xt16[:], xt[:])
            xT = xpool.tile([P, C * E], F16, tag="xT", name=f"xT{t}")
            for g, (c0, nch) in enumerate(((0, 8), (8, 8), (16, 2))):
                pt = tpp.tile([P, 1024], F16, tag="pt", name=f"pt{t}_{g}")
                for a in range(nch):
                    j = c0 + a
                    nc.tensor.transpose(pt[:, P * a:P * (a + 1)],
                                        xt16[:, P * j:P * (j + 1)], idt[:])
                job += 1
                if (job * DVE_SHARE) % 1.0 < DVE_SHARE:
                    vec.tensor_copy(xT[:, P * c0:P * (c0 + nch)],
                                    pt[:, :P * nch])
                else:
                    act.copy(xT[:, P * c0:P * (c0 + nch)], pt[:, :P * nch])
            pc = pcp.tile([P, 18], f32, tag="pc", name=f"pc{t}")
            for j in range(NCH):
                nc.tensor.matmul(pc[:], xT[:, P * j:P * (j + 1)],
                                 v(wm_sb, 18 * j, (1, 18)),
                                 start=(j == 0), stop=(j == NCH - 1))
            pcs = xpool.tile([P, 18], f32, tag="pcs", name=f"pcs{t}")
            act.copy(pcs[:], pc[:])
            vec.tensor_tensor(v(cam, t * E, (1, E)), pcs[:, 0:E], pcs[:, E:18],
                              ALU.add)

        # ---- SO(3) projection ---------------------------------------------
        NE = TPC * E                         # 288

        def mat(tile, off=0):
            # [P, (TPC, 3, 3)] compact view with offset into each 9-block
            return v(tile, off, (E, TPC), (3, 3), (1, 3))

        def flat(tile):
            return v(tile, 0, (1, NE))

        def row0(tile):
            return v(tile, 0, (E, TPC), (1, 3))

        def diag(tile):
            return v(tile, 0, (E, TPC), (4, 3))

        def pl(tile):
            return v(tile, 0, (1, TPC))

        def bc9(tile):
            # [P, TPC] plane broadcast over the 9 entries of each matrix
            return v(tile, 0, (1, TPC), (0, E))

        def bc3(tile):
            return v(tile, 0, (1, TPC), (0, 3))

        _consts = {}

        def cb(val):
            # [P, 1] constant tile for activation bias operands
            if val not in _consts:
                ct = wp.tile([P, 1], f32, name=f"const{len(_consts)}")
                vec.memset(ct[:], float(val))
                _consts[val] = ct[:]
            return _consts[val]

        Ya = wp.tile([P, NE], f32)
        Yb = wp.tile([P, NE], f32)
        D = wp.tile([P, TPC * 36], f32)
        Cf = wp.tile([P, NE], f32)
        t1 = wp.tile([P, NE], f32)
        t2 = wp.tile([P, NE], f32)
        t3 = wp.tile([P, NE], f32)
        td = wp.tile([P, TPC * 3], f32)
        det = wp.tile([P, TPC], f32)
        det0 = wp.tile([P, TPC], f32)
        s1p = wp.tile([P, TPC], f32)
        s2p = wp.tile([P, TPC], f32)
        s3p = wp.tile([P, TPC], f32)
        s4p = wp.tile([P, TPC], f32)
        u1 = wp.tile([P, TPC], f32)
        u2 = wp.tile([P, TPC], f32)
        u3 = wp.tile([P, TPC], f32)
        u4 = wp.tile([P, TPC], f32)

        def dblock(off):
            # view of D selecting D[a_block, b_block] as (TPC, 3, 3)
            return v(D, off, (36, TPC), (6, 3), (1, 3))

        def build_D(Y):
            # D[m] = [[Y, Y], [Y, Y]] as a 6x6 (row-major, stride 6)
            src = v(Y, 0, (E, TPC), (3, 3), (1, 3))
            for off in (0, 3, 18, 21):
                act.copy(v(D, off, (36, TPC), (6, 3), (1, 3)), src)

        def cofactor(Y, out):
            # out[i,j] = D[i+1,j+1]D[i+2,j+2] - D[i+1,j+2]D[i+2,j+1]
            build_D(Y)
            vec.tensor_tensor(mat(t1), dblock(7), dblock(14), ALU.mult)
            vec.tensor_tensor(mat(t2), dblock(8), dblock(13), ALU.mult)
            vec.tensor_tensor(mat(out), mat(t1), mat(t2), ALU.subtract)

        def det_of(Y, Cof, out):
            vec.tensor_tensor(v(td, 0, (3, TPC), (1, 3)), row0(Y), row0(Cof),
                              ALU.mult)
            vec.tensor_reduce(pl(out), v(td, 0, (3, TPC), (1, 3)),
                              mybir.AxisListType.X, ALU.add)

        if STAGE <= 2 or STAGE in (15, 16):
            nc.sync.dma_start(out=y_flat, in_=flat(cam))
            return

        # Newton polar iteration
        Y = cam
        other = [Ya, Yb]
        for it in range(min(NEWTON_ITERS, 99 if STAGE > 3 else 1)):
            cofactor(Y, Cf)
            det_of(Y, Cf, det)
            if it == 0:
                vec.tensor_copy(pl(det0), pl(det))
            Yn = other[it % 2]
            if it < SCALED_ITERS:
                # mu = |det|^(-1/3) = exp(-ln(det^2 + eps)/6)
                vec.tensor_tensor(pl(s1p), pl(det), pl(det), ALU.mult)
                act.activation(pl(s1p), pl(s1p), ACT.Ln, bias=cb(1e-35))
                act.activation(pl(s1p), pl(s1p), ACT.Exp, scale=-1.0 / 6.0, bias=cb(0.0))
                # s = 0.5/(mu*det);  muh = 0.5*mu
                vec.tensor_tensor(pl(s2p), pl(s1p), pl(det), ALU.mult)
                vec.reciprocal(pl(s2p), pl(s2p))
                vec.tensor_scalar_mul(pl(s2p), pl(s2p), 0.5)
                vec.tensor_scalar_mul(pl(s1p), pl(s1p), 0.5)
                vec.tensor_tensor(flat(t1), flat(Y), bc9(s1p), ALU.mult)
                vec.tensor_tensor(flat(t2), flat(Cf), bc9(s2p), ALU.mult)
                vec.tensor_tensor(flat(Yn), flat(t1), flat(t2), ALU.add)
            else:
                vec.reciprocal(pl(s2p), pl(det))
                vec.tensor_scalar_mul(pl(s2p), pl(s2p), 0.5)
                vec.tensor_scalar_mul(flat(t1), flat(Y), 0.5)
                vec.tensor_tensor(flat(t2), flat(Cf), bc9(s2p), ALU.mult)
                vec.tensor_tensor(flat(Yn), flat(t1), flat(t2), ALU.add)
            Y = Yn
        orth = Y

        if STAGE <= 4:
            nc.sync.dma_start(out=y_flat, in_=flat(orth))
            return

        # ---- reflection correction ---------------------------------------
        # P = orth^T @ cam  (into t3)
        Pm = t3
        for k in range(3):
            a = v(orth, 3 * k, (E, TPC), (1, 3), (0, 3))
            b = v(cam, 3 * k, (E, TPC), (0, 3), (1, 3))
            if k == 0:
                vec.tensor_tensor(mat(Pm), a, b, ALU.mult)
            else:
                vec.tensor_tensor(mat(t1), a, b, ALU.mult)
                vec.tensor_tensor(mat(Pm), mat(Pm), mat(t1), ALU.add)

        cofactor(Pm, Cf)                      # CP in Cf (uses t1, t2)
        c2 = s1p
        c1 = s2p
        c0 = s3p
        vec.tensor_reduce(pl(c2), diag(Pm), mybir.AxisListType.X, ALU.add)
        vec.tensor_reduce(pl(c1), diag(Cf), mybir.AxisListType.X, ALU.add)
        det_of(Pm, Cf, c0)

        q = det                               # reuse (det0 still holds sign info)
        p26 = wp.tile([P, TPC], f32)
        pp = wp.tile([P, TPC], f32)
        r = wp.tile([P, TPC], f32)
        s3 = s4p
        vec.tensor_scalar_mul(pl(q), pl(c2), 1.0 / 3.0)
        # p2/6 = ((2/3)c2^2 - 2 c1)/6 = c2^2/9 - c1/3
        vec.tensor_scalar_mul(pl(r), pl(c1), -1.0 / 3.0)
        vec.tensor_tensor(pl(p26), pl(c2), pl(c2), ALU.mult)
        vec.tensor_scalar_mul(pl(p26), pl(p26), 1.0 / 9.0)
        vec.tensor_tensor(pl(p26), pl(p26), pl(r), ALU.add)
        vec.tensor_scalar(pl(p26), pl(p26), 0.0, None, ALU.max)
        act.activation(pl(pp), pl(p26), ACT.Sqrt, bias=cb(1e-30))
        # detB = ((c2 - q)q - c1)q + c0 ; (c2 - q) = (2/3) c2
        vec.tensor_scalar_mul(pl(r), pl(c2), 2.0 / 3.0)
        vec.tensor_tensor(pl(r), pl(r), pl(q), ALU.mult)
        vec.tensor_tensor(pl(r), pl(r), pl(c1), ALU.subtract)
        vec.tensor_tensor(pl(r), pl(r), pl(q), ALU.mult)
        vec.tensor_tensor(pl(r), pl(r), pl(c0), ALU.add)     # r := detB
        # r = detB / (2 p^3 + eps)
        p3 = p26
        vec.tensor_tensor(pl(p3), pl(p26), pl(pp), ALU.mult)
        vec.tensor_scalar(pl(p3), pl(p3), 2.0, 1e-30, ALU.mult, ALU.add)
        vec.reciprocal(pl(p3), pl(p3))
        vec.tensor_tensor(pl(r), pl(r), pl(p3), ALU.mult)
        vec.tensor_scalar(pl(r), pl(r), -1.0, 1.0, ALU.max, ALU.min)
        # acos(r) via range-reduced atan (HW atan domain is [-pi/2, pi/2]):
        #   u = sqrt(1-r^2); phi = atan(min(|r|,u)/max(|r|,u)) in [0, pi/4]
        #   acos(r) = A + B*phi, A = pi/2*(1 - g + 2 s g), B = (1-2s)(2g-1)
        #   g = (|r| > u), s = (r < 0)
        vec.tensor_tensor(pl(u1), pl(r), pl(r), ALU.mult)
        act.activation(pl(u1), pl(u1), ACT.Sqrt, scale=-1.0, bias=cb(1.0 + 1e-12))
        vec.tensor_scalar_mul(pl(u2), pl(r), -1.0)
        vec.tensor_tensor(pl(u2), pl(u2), pl(r), ALU.max)         # |r|
        vec.tensor_tensor(pl(u3), pl(u2), pl(u1), ALU.min)
        vec.tensor_tensor(pl(u4), pl(u2), pl(u1), ALU.max)
        vec.reciprocal(pl(u4), pl(u4))
        vec.tensor_tensor(pl(u3), pl(u3), pl(u4), ALU.mult)
        act.activation(pl(u3), pl(u3), ACT.Arctan, bias=cb(0.0))
        zb = v(cb(0.0), 0, (0, TPC))
        vec.tensor_tensor(pl(u4), pl(u2), pl(u1), ALU.is_gt)      # g
        vec.tensor_tensor(pl(u2), pl(r), zb, ALU.is_lt)           # s
        vec.tensor_tensor(pl(u1), pl(u2), pl(u4), ALU.mult)       # s*g
        vec.tensor_scalar(pl(u1), pl(u1), np.pi, None, ALU.mult)
        vec.tensor_scalar(pl(r), pl(u4), -np.pi / 2.0, np.pi / 2.0,
                          ALU.mult, ALU.add)
        vec.tensor_tensor(pl(u1), pl(u1), pl(r), ALU.add)         # A
        vec.tensor_scalar(pl(u2), pl(u2), -2.0, 1.0, ALU.mult, ALU.add)
        vec.tensor_scalar(pl(u4), pl(u4), 2.0, -1.0, ALU.mult, ALU.add)
        vec.tensor_tensor(pl(u2), pl(u2), pl(u4), ALU.mult)       # B
        vec.tensor_tensor(pl(u3), pl(u3), pl(u2), ALU.mult)       # B*phi
        vec.tensor_tensor(pl(u1), pl(u1), pl(u3), ALU.add)        # acos(r)
        # s3 = q - 2 p sin(acos/3 + pi/6)   (== q + 2p cos(acos/3 + 2pi/3))
        act.activation(pl(u1), pl(u1), ACT.Sin, scale=1.0 / 3.0, bias=cb(np.pi / 6.0))
        vec.tensor_tensor(pl(u1), pl(pp), pl(u1), ALU.mult)
        vec.scalar_tensor_tensor(pl(s3), pl(u1), -2.0, pl(q), ALU.mult, ALU.add)

        # Newton polish on p(l) = -l^3 + c2 l^2 - c1 l + c0
        plv = pp
        dpl = r
        for _ in range(POLISH_ITERS):
            vec.tensor_tensor(pl(plv), pl(c2), pl(s3), ALU.subtract)
            vec.tensor_tensor(pl(plv), pl(plv), pl(s3), ALU.mult)
            vec.tensor_tensor(pl(plv), pl(plv), pl(c1), ALU.subtract)
            vec.tensor_tensor(pl(plv), pl(plv), pl(s3), ALU.mult)
            vec.tensor_tensor(pl(plv), pl(plv), pl(c0), ALU.add)
            vec.tensor_scalar(pl(dpl), pl(s3), -3.0, None, ALU.mult)
            vec.scalar_tensor_tensor(pl(dpl), pl(c2), 2.0, pl(dpl),
                                     ALU.mult, ALU.add)
            vec.tensor_tensor(pl(dpl), pl(dpl), pl(s3), ALU.mult)
            vec.tensor_tensor(pl(dpl), pl(dpl), pl(c1), ALU.subtract)
            vec.tensor_scalar(pl(dpl), pl(dpl), -1e-20, None, ALU.add)
            vec.reciprocal(pl(dpl), pl(dpl))
            vec.tensor_tensor(pl(plv), pl(plv), pl(dpl), ALU.mult)
            vec.tensor_tensor(pl(s3), pl(s3), pl(plv), ALU.subtract)

        # Nadj = CP + s3*P + (s3^2 - s3*c2) I
        w1 = q
        vec.tensor_tensor(pl(w1), pl(s3), pl(c2), ALU.mult)
        vec.tensor_tensor(pl(plv), pl(s3), pl(s3), ALU.mult)
        vec.tensor_tensor(pl(w1), pl(plv), pl(w1), ALU.subtract)
        vec.tensor_tensor(flat(t1), flat(Pm), bc9(s3), ALU.mult)
        vec.tensor_tensor(flat(Cf), flat(Cf), flat(t1), ALU.add)
        vec.tensor_tensor(diag(Cf), diag(Cf), bc3(w1), ALU.add)
        # proj = Nadj / (tr + eps)
        vec.tensor_reduce(pl(plv), diag(Cf), mybir.AxisListType.X, ALU.add)
        vec.tensor_scalar(pl(plv), pl(plv), 1e-30, None, ALU.add)
        vec.reciprocal(pl(plv), pl(plv))
        vec.tensor_tensor(flat(Cf), flat(Cf), bc9(plv), ALU.mult)
        # corr = orth @ proj
        corr = t3                              # Pm no longer needed
        for k in range(3):
            a = v(orth, k, (E, TPC), (3, 3), (0, 3))
            b = v(Cf, 3 * k, (E, TPC), (0, 3), (1, 3))
            if k == 0:
                vec.tensor_tensor(mat(corr), a, b, ALU.mult)
            else:
                vec.tensor_tensor(mat(t1), a, b, ALU.mult)
                vec.tensor_tensor(mat(corr), mat(corr), mat(t1), ALU.add)
        # f = 2*(det0 < 0);  R = orth - clamp(f*corr)
        vec.tensor_tensor(pl(plv), pl(det0), v(cb(0.0), 0, (0, TPC)), ALU.is_lt)
        vec.tensor_scalar_mul(pl(plv), pl(plv), 2.0)
        vec.tensor_tensor(flat(corr), flat(corr), bc9(plv), ALU.mult)
        vec.tensor_scalar(flat(corr), flat(corr), -2.0, 2.0, ALU.max, ALU.min)
        vec.tensor_tensor(flat(t1), flat(orth), flat(corr), ALU.subtract)

        nc.sync.dma_start(out=y_flat, in_=flat(t1))


def build(b_local=B_LOCAL):
    global TPC
    TPC = b_local // P
    nc = bacc.Bacc("TRN2", target_bir_lowering=False, debug=False)
    x = nc.dram_tensor("x", [b_local, C, 3, 3], F32, kind="ExternalInput")
    w = nc.dram_tensor("W", [C], F32, kind="ExternalInput")
    wm = nc.dram_tensor("wm", [18, P, 18], mybir.dt.float16, kind="ExternalInput")
    y = nc.dram_tensor("y", [b_local, 3, 3], F32, kind="ExternalOutput")
    with TileContext(nc) as tc:
        _emit(nc, tc, x.ap(), w.ap(), wm.ap(), y.ap())
    nc.compile()
    return nc


_NC_CACHE = {}


def kernel(x: np.ndarray, W: np.ndarray) -> np.ndarray:
    assert x.shape == (B_FULL, C, 3, 3) and W.shape == (C,)
    if "nc" not in _NC_CACHE:
        _NC_CACHE["nc"] = build()
    nc = _NC_CACHE["nc"]
    xs = np.ascontiguousarray(x.reshape(N_CORES, B_LOCAL, C, 3, 3))
    wm = make_wm(np.asarray(W, dtype=np.float32))
    in_maps = [{"x": xs[i], "W": W, "wm": wm} for i in range(N_CORES)]
    res = bass_utils.run_bass_kernel_spmd(nc, in_maps, core_ids=list(range(N_CORES)))
    return np.concatenate([r["y"] for r in res.results], axis=0)


if __name__ == "__main__":
    rng = np.random.default_rng(0)
    x = rng.standard_normal((B_FULL, C, 3, 3), dtype=np.float32)
    W = (rng.standard_normal(C, dtype=np.float32) / np.sqrt(C)).astype(np.float32)
    out = kernel(x=x, W=W)
    print(out.shape, out.dtype)



# revision 6
# speedup vs baseline: 1.1998x; 1.1998x over previous
"""Trainium2 Bass kernel for nn_CameraEstimator.

Per batch item b:
    camera[b] = einsum('chw,c->hw', x[b], W)     (C=256 contraction)
    out[b]    = nearest-rotation(camera[b])      (SVD u@vh + det reflection fix)

Contraction: x is cast to fp16 (ACT) and contracted on DVE (fp16 4x mode:
elementwise mult by the replicated fp16 W, in-place halving fold tree, then a
grouped fp32 reduce) -> camera [128, TPC*9] in SBUF.

SO(3) projection: closed form via the symmetric eigenproblem of K = Y^T Y.
Eigenvalues from the trig formula (acos via range-reduced atan) + one Newton
polish; then R = Y * f(K) where f(K) = g1*I + d12*(K-l1 I) + d123*(K-l1 I)(K-l2 I)
is the quadratic matrix interpolant of g(l_i) = sigma_i/sqrt(l_i) with
divided-difference coefficients (sigma_3 = sign(det Y) implements the
reflection fix; stable closed forms for det>0, explicit differences otherwise).

Sharding: batch dim split across 8 NeuronCores (data parallel), W replicated.
"""

import os
import numpy as np

import concourse.bacc as bacc
import concourse.bass as bass
import concourse.mybir as mybir
from concourse.bass_types import AP
from concourse.tile import TileContext
from concourse import bass_utils

F32 = mybir.dt.float32
F16 = mybir.dt.float16
ALU = mybir.AluOpType
ACT = mybir.ActivationFunctionType
AX = mybir.AxisListType

B_FULL = 32768
C = 256
E = 9
CE = C * E                           # 2304
N_CORES = 8
P = 128
B_LOCAL = B_FULL // N_CORES          # 4096
TPC = B_LOCAL // P                   # 32 matrices per partition

# (t0, ntiles) DMA chunks; tail chunks shrink so the last cast/contract is short
CHUNKS = [(0, 4), (4, 4), (8, 4), (12, 4), (16, 4), (20, 4), (24, 4),
          (28, 2), (30, 1), (31, 1)]
MAXNT = max(nt for _, nt in CHUNKS)
# tail groups (t0, ntiles): group 0 overlaps the tail of the load phase
GROUPS = [(0, 16), (16, 16)]
POLISH2 = True                       # also polish lambda_2 (1 Newton step)


def v(base: AP, off: int, *dims) -> AP:
    """Free-dim view of a tile AP: keep partition dim, set free dims.

    dims are (stride, count) pairs in element units relative to the tile row.
    """
    return AP(base.tensor, base.offset + off,
              [list(base.ap[0])] + [[s, c] for (s, c) in dims])


def make_w16(W: np.ndarray) -> np.ndarray:
    """fp16 W replicated over the 9 matrix entries: w16[c*9+e] = fp16(W[c])."""
    return np.ascontiguousarray(np.repeat(W.astype(np.float16), E))


def _emit(nc, tc, x_ap, w16_ap, y_ap):
    f32 = F32
    vec = nc.vector
    act = nc.scalar
    STAGE = int(os.environ.get("KERNEL_STAGE", "99"))

    # b = p*TPC + t so per-partition chunk loads/stores are contiguous
    x_flat = x_ap.rearrange("b c h w -> b (c h w)")
    x_tiled = x_flat.rearrange("(p t) f -> p t f", p=P)
    y_flat = y_ap.rearrange("b h w -> b (h w)").rearrange("(p t) e -> p (t e)", p=P)

    with tc.tile_pool(name="xin", bufs=2) as xpool, \
         tc.tile_pool(name="wk", bufs=1) as wp, \
         tc.tile_pool(name="tail", bufs=2) as tp:

        # ---- replicated fp16 weights -------------------------------------
        w16 = wp.tile([P, CE], F16)
        nc.sync.dma_start(out=w16[:1, :],
                          in_=AP(w16_ap.tensor, 0, [[1, 1], [1, CE]]))
        nc.gpsimd.partition_broadcast(w16[:], w16[:1, :])

        cam = wp.tile([P, TPC * E], f32)     # fp32 cameras, (t, e) compact

        _consts = {}

        def cb(val):
            if val not in _consts:
                ct = wp.tile([P, 1], f32, name=f"const{len(_consts)}")
                vec.memset(ct[:], float(val))
                _consts[val] = ct[:]
            return _consts[val]

        # ---- contraction: cast (ACT) + fp16 mult/fold tree (DVE) ---------
        for (t0, nt) in CHUNKS:
            ncol = nt * CE
            xt = xpool.tile([P, MAXNT * CE], f32, tag="xt", name=f"xt{t0}")
            nc.sync.dma_start(out=xt[:, :ncol], in_=x_tiled[:, t0:t0 + nt, :])
            x16 = xpool.tile([P, MAXNT * CE], F16, tag="x16", name=f"x16_{t0}")
            act.copy(x16[:, :ncol], xt[:, :ncol])
            # prod = x16 * w16 (w16 broadcast across the nt tiles), in place
            vec.tensor_tensor(v(x16, 0, (CE, nt), (1, CE)),
                              v(x16, 0, (CE, nt), (1, CE)),
                              v(w16, 0, (0, nt), (1, CE)), ALU.mult)
            n = CE
            while n > 72:                    # halving folds over c, in place
                n //= 2
                vec.tensor_tensor(v(x16, 0, (CE, nt), (1, n)),
                                  v(x16, 0, (CE, nt), (1, n)),
                                  v(x16, n, (CE, nt), (1, n)), ALU.add)
            for i in range(nt):              # grouped 72 -> 9 fp32 reduce
                t = t0 + i
                vec.tensor_reduce(v(cam, t * E, (1, E)),
                                  v(x16, i * CE, (1, E), (E, 8)),
                                  AX.X, ALU.add)

        if STAGE <= 2:
            nc.sync.dma_start(out=y_flat, in_=v(cam, 0, (1, TPC * E)))
            return

        # ---- closed-form SO(3) projection per group ----------------------
        for gi, (t0, G) in enumerate(GROUPS):
            NE = G * E

            def mat(tile, off=0):
                return v(tile, off, (E, G), (3, 3), (1, 3))

            def flat(tile):
                return v(tile, 0, (1, NE))

            def diag(tile):
                return v(tile, 0, (E, G), (4, 3))

            def pl(tile):
                return v(tile, 0, (1, G))

            def bc9(tile):
                return v(tile, 0, (1, G), (0, E))

            def bc3(tile):
                return v(tile, 0, (1, G), (0, 3))

            camg = v(cam, t0 * E, (1, NE))   # this group's cameras, flat

            def cmat(off=0):
                return v(cam, t0 * E + off, (E, G), (3, 3), (1, 3))

            def T(name, cols):
                return tp.tile([P, cols], f32, tag=name, name=f"{name}_{gi}")

            K = T("K", NE)
            Bm = T("Bm", NE)
            Mt = T("Mt", NE)
            t1 = T("t1", NE)
            Rt = T("Rt", NE)
            D2 = T("D2", G * 12)             # rows 1,2 of Y duplicated as 2x6

            def plane(name):
                return T("p_" + name, G)

            c2 = plane("c2"); c2sq = plane("c2sq"); k2 = plane("k2")
            c1 = plane("c1"); c0 = plane("c0"); dety = plane("dety")
            q = plane("q"); p2 = plane("p2"); pp = plane("pp")
            detb = plane("detb"); r = plane("r"); w1 = plane("w1")
            w2 = plane("w2"); w3 = plane("w3"); w4 = plane("w4")
            th = plane("th"); l1 = plane("l1"); l2 = plane("l2"); l3 = plane("l3")
            a1 = plane("a1"); a2 = plane("a2"); a3 = plane("a3")
            al1 = plane("al1"); al2 = plane("al2"); al3 = plane("al3")
            d12 = plane("d12"); d23 = plane("d23"); d123 = plane("d123")
            sneg = plane("sneg"); td = T("p_td", 6 * G)

            zb = v(cb(0.0), 0, (0, G))

            # K = Y^T Y: sum_k outer(row_k, row_k)
            for k in range(3):
                a = v(cam, t0 * E + 3 * k, (E, G), (1, 3), (0, 3))
                b = v(cam, t0 * E + 3 * k, (E, G), (0, 3), (1, 3))
                if k == 0:
                    vec.tensor_tensor(mat(K), a, b, ALU.mult)
                else:
                    vec.tensor_tensor(mat(t1), a, b, ALU.mult)
                    vec.tensor_tensor(mat(K), mat(K), mat(t1), ALU.add)

            # c2 = tr K ; c1 = (c2^2 - ||K||_F^2)/2
            vec.tensor_reduce(pl(c2), diag(K), AX.X, ALU.add)
            vec.tensor_tensor(flat(t1), flat(K), flat(K), ALU.mult)
            vec.tensor_reduce(pl(k2), v(t1, 0, (E, G), (1, E)), AX.X, ALU.add)
            vec.tensor_tensor(pl(c2sq), pl(c2), pl(c2), ALU.mult)
            vec.tensor_tensor(pl(c1), pl(c2sq), pl(k2), ALU.subtract)
            vec.tensor_scalar_mul(pl(c1), pl(c1), 0.5)

            # det Y via duplicated rows 1,2: cross(r1, r2) . r0
            for rep in range(2):
                act.copy(v(D2, 3 * rep, (12, G), (6, 2), (1, 3)),
                         v(cam, t0 * E + 3, (E, G), (3, 2), (1, 3)))
            # cross[j] = D2[0, j+1]*D2[1, j+2] - D2[0, j+2]*D2[1, j+1]
            vec.tensor_tensor(v(td, 0, (3, G), (1, 3)),
                              v(D2, 1, (12, G), (1, 3)),
                              v(D2, 8, (12, G), (1, 3)), ALU.mult)
            vec.tensor_tensor(v(td, 3 * G, (3, G), (1, 3)),
                              v(D2, 2, (12, G), (1, 3)),
                              v(D2, 7, (12, G), (1, 3)), ALU.mult)
            vec.tensor_tensor(v(td, 0, (3, G), (1, 3)),
                              v(td, 0, (3, G), (1, 3)),
                              v(td, 3 * G, (3, G), (1, 3)), ALU.subtract)
            vec.tensor_tensor(v(td, 0, (3, G), (1, 3)),
                              v(td, 0, (3, G), (1, 3)),
                              v(cam, t0 * E, (E, G), (1, 3)), ALU.mult)
            vec.tensor_reduce(pl(dety), v(td, 0, (3, G), (1, 3)), AX.X, ALU.add)
            vec.tensor_tensor(pl(c0), pl(dety), pl(dety), ALU.mult)

            # eigenvalues via trig formula
            vec.tensor_scalar_mul(pl(q), pl(c2), 1.0 / 3.0)
            vec.tensor_scalar_mul(pl(w1), pl(c1), -1.0 / 3.0)
            vec.tensor_scalar(pl(p2), pl(c2sq), 1.0 / 9.0, None, ALU.mult)
            vec.tensor_tensor(pl(p2), pl(p2), pl(w1), ALU.add)
            vec.tensor_scalar(pl(p2), pl(p2), 1e-30, None, ALU.max)
            act.activation(pl(pp), pl(p2), ACT.Sqrt, bias=cb(0.0))     # p
            # detB = ((c2-q)q - c1)q + c0, with (c2-q) = (2/3)c2
            vec.tensor_scalar_mul(pl(w1), pl(c2), 2.0 / 3.0)
            vec.tensor_tensor(pl(w1), pl(w1), pl(q), ALU.mult)
            vec.tensor_tensor(pl(w1), pl(w1), pl(c1), ALU.subtract)
            vec.tensor_tensor(pl(w1), pl(w1), pl(q), ALU.mult)
            vec.tensor_tensor(pl(detb), pl(w1), pl(c0), ALU.add)
            # r = detB / (2 p^3), clamped to [-1, 1]
            vec.tensor_tensor(pl(w1), pl(pp), pl(p2), ALU.mult)        # p^3
            vec.tensor_scalar(pl(w1), pl(w1), 2.0, 1e-30, ALU.mult, ALU.add)
            vec.reciprocal(pl(w1), pl(w1))
            vec.tensor_tensor(pl(r), pl(detb), pl(w1), ALU.mult)
            vec.tensor_scalar(pl(r), pl(r), -1.0, 1.0, ALU.max, ALU.min)
            # acos(r) via range-reduced atan:
            #   u = sqrt(1-r^2); phi = atan(min(|r|,u)/max(|r|,u)) in [0, pi/4]
            #   acos(r) = A + B*phi, A = pi/2*(1 - g + 2 s g), B = (1-2s)(2g-1)
            vec.tensor_tensor(pl(w1), pl(r), pl(r), ALU.mult)
            act.activation(pl(w1), pl(w1), ACT.Sqrt, scale=-1.0,
                           bias=cb(1.0 + 1e-12))                       # u
            vec.tensor_scalar_mul(pl(w2), pl(r), -1.0)
            vec.tensor_tensor(pl(w2), pl(w2), pl(r), ALU.max)          # |r|
            vec.tensor_tensor(pl(w3), pl(w2), pl(w1), ALU.min)
            vec.tensor_tensor(pl(w4), pl(w2), pl(w1), ALU.max)
            vec.reciprocal(pl(w4), pl(w4))
            vec.tensor_tensor(pl(w3), pl(w3), pl(w4), ALU.mult)
            act.activation(pl(w3), pl(w3), ACT.Arctan, bias=cb(0.0))   # phi
            vec.tensor_tensor(pl(w4), pl(w2), pl(w1), ALU.is_gt)       # g
            vec.tensor_tensor(pl(w2), pl(r), zb, ALU.is_lt)            # s
            vec.tensor_tensor(pl(w1), pl(w2), pl(w4), ALU.mult)        # s*g
            vec.tensor_scalar(pl(w1), pl(w1), np.pi, None, ALU.mult)
            vec.tensor_scalar(pl(th), pl(w4), -np.pi / 2.0, np.pi / 2.0,
                              ALU.mult, ALU.add)
            vec.tensor_tensor(pl(w1), pl(w1), pl(th), ALU.add)         # A
            vec.tensor_scalar(pl(w2), pl(w2), -2.0, 1.0, ALU.mult, ALU.add)
            vec.tensor_scalar(pl(w4), pl(w4), 2.0, -1.0, ALU.mult, ALU.add)
            vec.tensor_tensor(pl(w2), pl(w2), pl(w4), ALU.mult)        # B
            vec.tensor_tensor(pl(w3), pl(w3), pl(w2), ALU.mult)
            vec.tensor_tensor(pl(th), pl(w1), pl(w3), ALU.add)         # acos
            # l1 = q + 2p sin(pi/2 - th/3); l2 = q + 2p sin(th/3 - pi/6);
            # l3 = q - 2p sin(th/3 + pi/6)
            vec.tensor_scalar_mul(pl(w4), pl(pp), 2.0)                 # 2p
            act.activation(pl(w1), pl(th), ACT.Sin, scale=-1.0 / 3.0,
                           bias=cb(np.pi / 2.0))
            act.activation(pl(w2), pl(th), ACT.Sin, scale=1.0 / 3.0,
                           bias=cb(-np.pi / 6.0))
            act.activation(pl(w3), pl(th), ACT.Sin, scale=1.0 / 3.0,
                           bias=cb(np.pi / 6.0))
            vec.tensor_tensor(pl(w1), pl(w1), pl(w4), ALU.mult)
            vec.tensor_tensor(pl(l1), pl(w1), pl(q), ALU.add)
            vec.tensor_tensor(pl(w2), pl(w2), pl(w4), ALU.mult)
            vec.tensor_tensor(pl(l2), pl(w2), pl(q), ALU.add)
            vec.tensor_tensor(pl(w3), pl(w3), pl(w4), ALU.mult)
            vec.tensor_tensor(pl(l3), pl(q), pl(w3), ALU.subtract)

            # one Newton polish on f(l) = -l^3 + c2 l^2 - c1 l + c0
            vec.tensor_scalar_mul(pl(c2sq), pl(c2), 2.0)               # 2c2
            def polish(l, guard):
                vec.tensor_tensor(pl(w1), pl(c2), pl(l), ALU.subtract)
                vec.tensor_tensor(pl(w1), pl(w1), pl(l), ALU.mult)
                vec.tensor_tensor(pl(w1), pl(w1), pl(c1), ALU.subtract)
                vec.tensor_tensor(pl(w1), pl(w1), pl(l), ALU.mult)
                vec.tensor_tensor(pl(w1), pl(w1), pl(c0), ALU.add)     # f
                vec.tensor_scalar(pl(w2), pl(l), -3.0, None, ALU.mult)
                vec.tensor_tensor(pl(w2), pl(w2), pl(c2sq), ALU.add)
                vec.tensor_tensor(pl(w2), pl(w2), pl(l), ALU.mult)
                vec.tensor_tensor(pl(w2), pl(w2), pl(c1), ALU.subtract)
                vec.tensor_scalar(pl(w2), pl(w2), guard, None, ALU.add)
                vec.reciprocal(pl(w2), pl(w2))
                vec.tensor_tensor(pl(w1), pl(w1), pl(w2), ALU.mult)
                vec.tensor_tensor(pl(l), pl(l), pl(w1), ALU.subtract)
            polish(l3, -1e-20)
            if POLISH2:
                polish(l2, 1e-20)
            vec.tensor_scalar(pl(l1), pl(l1), 1e-25, None, ALU.max)
            vec.tensor_scalar(pl(l2), pl(l2), 1e-25, None, ALU.max)
            vec.tensor_scalar(pl(l3), pl(l3), 1e-25, None, ALU.max)

            # alpha_i = sqrt(l_i), a_i = 1/alpha_i
            act.activation(pl(al1), pl(l1), ACT.Sqrt, bias=cb(0.0))
            act.activation(pl(al2), pl(l2), ACT.Sqrt, bias=cb(0.0))
            act.activation(pl(al3), pl(l3), ACT.Sqrt, bias=cb(0.0))
            vec.reciprocal(pl(a1), pl(al1))
            vec.reciprocal(pl(a2), pl(al2))
            vec.reciprocal(pl(a3), pl(al3))

            vec.tensor_tensor(pl(sneg), pl(dety), zb, ALU.is_lt)

            # d12 = -a1 a2 / (al1 + al2)
            vec.tensor_tensor(pl(w1), pl(al1), pl(al2), ALU.add)
            vec.reciprocal(pl(w1), pl(w1))
            vec.tensor_tensor(pl(w2), pl(a1), pl(a2), ALU.mult)
            vec.scalar_tensor_tensor(pl(d12), pl(w2), -1.0, pl(w1),
                                     ALU.mult, ALU.mult)
            # d23p = -a2 a3 / (al2 + al3); d23m = (a2 + a3) / (l2 - l3)
            vec.tensor_tensor(pl(w3), pl(al2), pl(al3), ALU.add)
            vec.reciprocal(pl(w4), pl(w3))
            vec.tensor_tensor(pl(w2), pl(a2), pl(a3), ALU.mult)
            vec.scalar_tensor_tensor(pl(d23), pl(w2), -1.0, pl(w4),
                                     ALU.mult, ALU.mult)               # d23p
            vec.tensor_tensor(pl(w2), pl(l2), pl(l3), ALU.subtract)
            vec.tensor_scalar(pl(w2), pl(w2), 1e-20, None, ALU.add)
            vec.reciprocal(pl(w2), pl(w2))
            vec.tensor_tensor(pl(w4), pl(a2), pl(a3), ALU.add)
            vec.tensor_tensor(pl(w4), pl(w4), pl(w2), ALU.mult)        # d23m
            vec.tensor_tensor(pl(w2), pl(w4), pl(d23), ALU.subtract)
            vec.tensor_tensor(pl(w2), pl(w2), pl(sneg), ALU.mult)
            vec.tensor_tensor(pl(d23), pl(d23), pl(w2), ALU.add)
            # d123p = (al1+al2+al3) / (al1 al2 al3 (al1+al2)(al2+al3)(al3+al1))
            # w1 still holds 1/(al1+al2) from the d12 block
            vec.tensor_tensor(pl(w2), pl(al3), pl(al1), ALU.add)
            vec.tensor_tensor(pl(w3), pl(w3), pl(w2), ALU.mult)        # (al2+al3)(al3+al1)
            vec.reciprocal(pl(w3), pl(w3))
            vec.tensor_tensor(pl(w2), pl(a1), pl(a2), ALU.mult)
            vec.tensor_tensor(pl(w2), pl(w2), pl(a3), ALU.mult)        # 1/(al1 al2 al3)
            vec.tensor_tensor(pl(w4), pl(al1), pl(al2), ALU.add)
            vec.tensor_tensor(pl(w4), pl(w4), pl(al3), ALU.add)        # S
            vec.tensor_tensor(pl(w4), pl(w4), pl(w1), ALU.mult)        # S/(al1+al2)
            vec.tensor_tensor(pl(w4), pl(w4), pl(w3), ALU.mult)
            vec.tensor_tensor(pl(d123), pl(w4), pl(w2), ALU.mult)      # d123p
            # d123m = (d12 - d23) / (l1 - l3)
            vec.tensor_tensor(pl(w2), pl(l1), pl(l3), ALU.subtract)
            vec.tensor_scalar(pl(w2), pl(w2), 1e-20, None, ALU.add)
            vec.reciprocal(pl(w2), pl(w2))
            vec.tensor_tensor(pl(w4), pl(d12), pl(d23), ALU.subtract)
            vec.tensor_tensor(pl(w4), pl(w4), pl(w2), ALU.mult)        # d123m
            vec.tensor_tensor(pl(w4), pl(w4), pl(d123), ALU.subtract)
            vec.tensor_tensor(pl(w4), pl(w4), pl(sneg), ALU.mult)
            vec.tensor_tensor(pl(d123), pl(d123), pl(w4), ALU.add)

            # A = K - l1 I (in place), Bm = K - l2 I
            vec.tensor_copy(flat(Bm), flat(K))
            vec.tensor_tensor(diag(K), diag(K), bc3(l1), ALU.subtract)
            vec.tensor_tensor(diag(Bm), diag(Bm), bc3(l2), ALU.subtract)
            # Mt = A @ Bm
            for k in range(3):
                a = v(K, k, (E, G), (3, 3), (0, 3))
                b = v(Bm, 3 * k, (E, G), (0, 3), (1, 3))
                if k == 0:
                    vec.tensor_tensor(mat(Mt), a, b, ALU.mult)
                else:
                    vec.tensor_tensor(mat(t1), a, b, ALU.mult)
                    vec.tensor_tensor(mat(Mt), mat(Mt), mat(t1), ALU.add)
            # Phi = d123*Mt + d12*A + a1*I  (accumulate into Mt)
            vec.tensor_tensor(flat(Mt), flat(Mt), bc9(d123), ALU.mult)
            vec.tensor_tensor(flat(t1), flat(K), bc9(d12), ALU.mult)
            vec.tensor_tensor(flat(Mt), flat(Mt), flat(t1), ALU.add)
            vec.tensor_tensor(diag(Mt), diag(Mt), bc3(a1), ALU.add)
            # R = Y @ Phi
            for k in range(3):
                a = v(cam, t0 * E + k, (E, G), (3, 3), (0, 3))
                b = v(Mt, 3 * k, (E, G), (0, 3), (1, 3))
                if k == 0:
                    vec.tensor_tensor(mat(Rt), a, b, ALU.mult)
                else:
                    vec.tensor_tensor(mat(t1), a, b, ALU.mult)
                    vec.tensor_tensor(mat(Rt), mat(Rt), mat(t1), ALU.add)

            nc.sync.dma_start(out=v(y_flat, t0 * E, (1, NE)), in_=flat(Rt))


def build(b_local=B_LOCAL):
    nc = bacc.Bacc("TRN2", target_bir_lowering=False, debug=False)
    x = nc.dram_tensor("x", [b_local, C, 3, 3], F32, kind="ExternalInput")
    w16 = nc.dram_tensor("w16", [CE], F16, kind="ExternalInput")
    y = nc.dram_tensor("y", [b_local, 3, 3], F32, kind="ExternalOutput")
    with TileContext(nc) as tc:
        _emit(nc, tc, x.ap(), w16.ap(), y.ap())
    nc.compile()
    return nc


_NC_CACHE = {}


def make_in_maps(x: np.ndarray, W: np.ndarray):
    xs = np.ascontiguousarray(x.reshape(N_CORES, B_LOCAL, C, 3, 3))
    w16 = make_w16(np.asarray(W, dtype=np.float32))
    return [{"x": xs[i], "w16": w16} for i in range(N_CORES)]


def kernel(x: np.ndarray, W: np.ndarray) -> np.ndarray:
    assert x.shape == (B_FULL, C, 3, 3) and W.shape == (C,)
    if "nc" not in _NC_CACHE:
        _NC_CACHE["nc"] = build()
    nc = _NC_CACHE["nc"]
    in_maps = make_in_maps(x, W)
    res = bass_utils.run_bass_kernel_spmd(nc, in_maps, core_ids=list(range(N_CORES)))
    return np.concatenate([r["y"] for r in res.results], axis=0)


if __name__ == "__main__":
    rng = np.random.default_rng(0)
    x = rng.standard_normal((B_FULL, C, 3, 3), dtype=np.float32)
    W = (rng.standard_normal(C, dtype=np.float32) / np.sqrt(C)).astype(np.float32)
    out = kernel(x=x, W=W)
    print(out.shape, out.dtype)


# revision 7
# speedup vs baseline: 1.9036x; 1.5866x over previous
"""Trainium2 Bass kernel for nn_CameraEstimator.

Per batch item b:
    camera[b] = einsum('chw,c->hw', x[b], W)     (C=256 contraction)
    out[b]    = nearest-rotation(camera[b])      (SVD u@vh + det reflection fix)

Contraction (hybrid, per tile of 128x2304):
  - DVE path: fp16 elementwise mult by replicated fp16 W + in-place halving
    fold tree (DVE 2x mode) + grouped fp32 reduce.
  - PE path: fp16 chunk transposes on the PE (identity matmul) -> PSUM,
    copyback (ACT/DVE), 18 accumulating matmuls against a masked fp16 W
    -> camera directly in PSUM.
  x arrives fp16 either via an ACT cast of the fp32 DMA, or (KERNEL_DMACAST=1)
  via a gpsimd SWDGE DMA that casts in flight.

SO(3) projection: closed form via the symmetric eigenproblem of K = Y^T Y.
Eigenvalues from the trig formula (acos via range-reduced atan) + one Newton
polish; then R = Y * f(K) with f(K) = g1*I + d12*(K-l1 I) + d123*(K-l1 I)(K-l2 I),
the quadratic matrix interpolant of g(l_i) = sigma_i/sqrt(l_i) via divided
differences (sigma_3 = sign(det Y) implements the reflection fix).
The K/det front block runs on the Pool engine, the rest on DVE/ACT.

Sharding: batch dim split across 8 NeuronCores (data parallel), W replicated.
"""

import os
import numpy as np

import concourse.bacc as bacc
import concourse.bass as bass
import concourse.mybir as mybir
from concourse.bass_types import AP
from concourse.tile import TileContext
from concourse import bass_utils

F32 = mybir.dt.float32
F16 = mybir.dt.float16
ALU = mybir.AluOpType
ACT = mybir.ActivationFunctionType
AX = mybir.AxisListType

B_FULL = 32768
C = 256
E = 9
CE = C * E                           # 2304
NCH = CE // 128                      # 18 128-wide chunks per tile
N_CORES = 8
P = 128
B_LOCAL = B_FULL // N_CORES          # 4096
TPC = B_LOCAL // P                   # 32 matrices per partition

CHUNKS = [(0, 4), (4, 4), (8, 4), (12, 4), (16, 4), (20, 4), (24, 4),
          (28, 2), (30, 1), (31, 1)]
MAXNT = max(nt for _, nt in CHUNKS)
GROUPS = [(0, 16), (16, 16)]

DMACAST = os.environ.get("KERNEL_DMACAST", "0") == "1"
PE_MOD = int(os.environ.get("KERNEL_PE_MOD", "2"))   # t%PE_MOD!=0 -> DVE path
POLISH2 = os.environ.get("KERNEL_POLISH2", "1") == "1"
POOL_FRONT = os.environ.get("KERNEL_POOL_FRONT", "1") == "1"


def is_pe_tile(t):
    return PE_MOD > 0 and (t % PE_MOD == 1)


def v(base: AP, off: int, *dims) -> AP:
    return AP(base.tensor, base.offset + off,
              [list(base.ap[0])] + [[s, c] for (s, c) in dims])


def make_w16(W: np.ndarray) -> np.ndarray:
    """fp16 W replicated over entries and partitions: [128, 2304]."""
    row = np.repeat(W.astype(np.float16), E)
    return np.ascontiguousarray(np.broadcast_to(row, (P, CE)))


def make_wm9(W: np.ndarray) -> np.ndarray:
    """Masked fp16 W for the PE path: wm9[j, k, e] = fp16(W[(128j+k)//9])
    where (128j+k) % 9 == e, else 0."""
    kidx = np.arange(CE)
    wm = np.zeros((CE, E), np.float32)
    wm[kidx, kidx % E] = W[kidx // E]
    return np.ascontiguousarray(wm.astype(np.float16).reshape(NCH, P, E))


def make_idt() -> np.ndarray:
    return np.ascontiguousarray(np.eye(P, dtype=np.float16))


def _emit(nc, tc, x_ap, w16_ap, wm9_ap, idt_ap, y_ap):
    f32 = F32
    vec = nc.vector
    act = nc.scalar
    pool = nc.gpsimd
    STAGE = int(os.environ.get("KERNEL_STAGE", "99"))

    x_flat = x_ap.rearrange("b c h w -> b (c h w)")
    x_tiled = x_flat.rearrange("(p t) f -> p t f", p=P)
    y_flat = y_ap.rearrange("b h w -> b (h w)").rearrange("(p t) e -> p (t e)", p=P)

    with tc.tile_pool(name="xin", bufs=2) as xpool, \
         tc.tile_pool(name="wk", bufs=1) as wp, \
         tc.tile_pool(name="xtp", bufs=2) as xtpool, \
         tc.tile_pool(name="tp", bufs=2, space="PSUM") as tpp, \
         tc.tile_pool(name="pcp", bufs=4, space="PSUM") as pcp, \
         tc.tile_pool(name="tail", bufs=2) as tp:

        # ---- persistent inputs ------------------------------------------
        w16 = wp.tile([P, CE], F16)
        nc.sync.dma_start(out=w16[:], in_=AP(w16_ap.tensor, 0, [[CE, P], [1, CE]]))
        wm9 = wp.tile([P, NCH * E], F16)
        nc.sync.dma_start(
            out=wm9[:],
            in_=AP(wm9_ap.tensor, 0, [[E, P], [E * P, NCH], [1, E]]))
        idt = wp.tile([P, P], F16)
        nc.sync.dma_start(out=idt[:], in_=AP(idt_ap.tensor, 0, [[P, P], [1, P]]))

        cam = wp.tile([P, TPC * E], f32)

        _consts = {}

        def cb(val):
            if val not in _consts:
                ct = wp.tile([P, 1], f32, name=f"const{len(_consts)}")
                vec.memset(ct[:], float(val))
                _consts[val] = ct[:]
            return _consts[val]

        # ---- contraction -------------------------------------------------
        cb_flip = [0]

        def emit_chunk(t0, nt):
            ncol = nt * CE
            if DMACAST:
                x16 = xpool.tile([P, MAXNT * CE], F16, tag="x16", name=f"x16_{t0}")
                nc.gpsimd.dma_start(out=x16[:, :ncol], in_=x_tiled[:, t0:t0 + nt, :])
            else:
                xt = xpool.tile([P, MAXNT * CE], F32, tag="xt", name=f"xt{t0}")
                nc.sync.dma_start(out=xt[:, :ncol], in_=x_tiled[:, t0:t0 + nt, :])
                x16 = xpool.tile([P, MAXNT * CE], F16, tag="x16", name=f"x16_{t0}")
                act.copy(x16[:, :ncol], xt[:, :ncol])

            dve_tiles = [i for i in range(nt) if not is_pe_tile(t0 + i)]
            pe_tiles = [i for i in range(nt) if is_pe_tile(t0 + i)]

            # DVE path: mult+folds in place on the dve tiles' regions
            for i in dve_tiles:
                off = i * CE
                vec.tensor_tensor(v(x16, off, (1, CE)), v(x16, off, (1, CE)),
                                  v(w16, 0, (1, CE)), ALU.mult)
                n = CE
                while n > 72:
                    n //= 2
                    vec.tensor_tensor(v(x16, off, (1, n)), v(x16, off, (1, n)),
                                      v(x16, off + n, (1, n)), ALU.add)
                t = t0 + i
                vec.tensor_reduce(v(cam, t * E, (1, E)),
                                  v(x16, off, (1, E), (E, 8)),
                                  AX.X, ALU.add)

            # PE path: transpose chunks -> PSUM -> copyback -> matmuls
            for i in pe_tiles:
                t = t0 + i
                off = i * CE
                xT = xtpool.tile([P, CE], F16, tag="xT", name=f"xT{t}")
                for g, (c0, nch) in enumerate(((0, 8), (8, 8), (16, 2))):
                    pt = tpp.tile([P, 1024], F16, tag="pt", name=f"pt{t}_{g}")
                    for a in range(nch):
                        j = c0 + a
                        nc.tensor.transpose(pt[:, P * a:P * (a + 1)],
                                            v(x16, off + P * j, (1, P)), idt[:])
                    if cb_flip[0] % 2 == 0:
                        act.copy(xT[:, P * c0:P * (c0 + nch)], pt[:, :P * nch])
                    else:
                        vec.tensor_copy(xT[:, P * c0:P * (c0 + nch)],
                                        pt[:, :P * nch])
                    cb_flip[0] += 1
                pc = pcp.tile([P, E], f32, tag="pc", name=f"pc{t}")
                for j in range(NCH):
                    nc.tensor.matmul(pc[:], xT[:, P * j:P * (j + 1)],
                                     wm9[:, E * j:E * (j + 1)],
                                     start=(j == 0), stop=(j == NCH - 1))
                act.copy(v(cam, t * E, (1, E)), pc[:, :])

        def emit_group(gi, t0, G):
            NE = G * E

            def mat(tile, off=0):
                return v(tile, off, (E, G), (3, 3), (1, 3))

            def flat(tile):
                return v(tile, 0, (1, NE))

            def diag(tile):
                return v(tile, 0, (E, G), (4, 3))

            def pl(tile):
                return v(tile, 0, (1, G))

            def bc9(tile):
                return v(tile, 0, (1, G), (0, E))

            def bc3(tile):
                return v(tile, 0, (1, G), (0, 3))

            def T(name, cols):
                return tp.tile([P, cols], f32, tag=name, name=f"{name}_{gi}")

            K = T("K", NE)
            Bm = T("Bm", NE)
            Mt = T("Mt", NE)
            t1 = T("t1", NE)
            Rt = T("Rt", NE)
            D2 = T("D2", G * 12)

            def plane(name):
                return T("p_" + name, G)

            c2 = plane("c2"); c2sq = plane("c2sq"); k2 = plane("k2")
            c1 = plane("c1"); c0 = plane("c0"); dety = plane("dety")
            q = plane("q"); p2 = plane("p2"); pp = plane("pp")
            detb = plane("detb"); r = plane("r"); w1 = plane("w1")
            w2 = plane("w2"); w3 = plane("w3"); w4 = plane("w4")
            th = plane("th"); l1 = plane("l1"); l2 = plane("l2"); l3 = plane("l3")
            a1 = plane("a1"); a2 = plane("a2"); a3 = plane("a3")
            al1 = plane("al1"); al2 = plane("al2"); al3 = plane("al3")
            d12 = plane("d12"); d23 = plane("d23"); d123 = plane("d123")
            sneg = plane("sneg"); td = T("p_td", 6 * G)

            zb = v(cb(0.0), 0, (0, G))
            fr = pool if POOL_FRONT else vec      # front-block engine

            # K = Y^T Y
            for k in range(3):
                a = v(cam, t0 * E + 3 * k, (E, G), (1, 3), (0, 3))
                b = v(cam, t0 * E + 3 * k, (E, G), (0, 3), (1, 3))
                if k == 0:
                    fr.tensor_tensor(mat(K), a, b, ALU.mult)
                else:
                    fr.tensor_tensor(mat(t1), a, b, ALU.mult)
                    fr.tensor_tensor(mat(K), mat(K), mat(t1), ALU.add)
            fr.tensor_reduce(pl(c2), diag(K), AX.X, ALU.add)
            fr.tensor_tensor(flat(t1), flat(K), flat(K), ALU.mult)
            fr.tensor_reduce(pl(k2), v(t1, 0, (E, G), (1, E)), AX.X, ALU.add)
            fr.tensor_tensor(pl(c2sq), pl(c2), pl(c2), ALU.mult)
            fr.tensor_tensor(pl(c1), pl(c2sq), pl(k2), ALU.subtract)
            fr.tensor_scalar_mul(pl(c1), pl(c1), 0.5)
            # det Y
            for rep in range(2):
                fr.tensor_copy(v(D2, 3 * rep, (12, G), (6, 2), (1, 3)),
                               v(cam, t0 * E + 3, (E, G), (3, 2), (1, 3)))
            fr.tensor_tensor(v(td, 0, (3, G), (1, 3)),
                             v(D2, 1, (12, G), (1, 3)),
                             v(D2, 8, (12, G), (1, 3)), ALU.mult)
            fr.tensor_tensor(v(td, 3 * G, (3, G), (1, 3)),
                             v(D2, 2, (12, G), (1, 3)),
                             v(D2, 7, (12, G), (1, 3)), ALU.mult)
            fr.tensor_tensor(v(td, 0, (3, G), (1, 3)),
                             v(td, 0, (3, G), (1, 3)),
                             v(td, 3 * G, (3, G), (1, 3)), ALU.subtract)
            fr.tensor_tensor(v(td, 0, (3, G), (1, 3)),
                             v(td, 0, (3, G), (1, 3)),
                             v(cam, t0 * E, (E, G), (1, 3)), ALU.mult)
            fr.tensor_reduce(pl(dety), v(td, 0, (3, G), (1, 3)), AX.X, ALU.add)
            fr.tensor_tensor(pl(c0), pl(dety), pl(dety), ALU.mult)
            fr.tensor_tensor(pl(sneg), pl(dety), zb, ALU.is_lt)

            # eigenvalues via trig formula (DVE planes + ACT)
            vec.tensor_scalar_mul(pl(q), pl(c2), 1.0 / 3.0)
            vec.tensor_scalar_mul(pl(w1), pl(c1), 1.0 / 3.0)
            vec.scalar_tensor_tensor(pl(p2), pl(c2sq), 1.0 / 9.0, pl(w1),
                                     ALU.mult, ALU.subtract)
            vec.tensor_scalar(pl(p2), pl(p2), 1e-30, None, ALU.max)
            act.activation(pl(pp), pl(p2), ACT.Sqrt, bias=cb(0.0))
            vec.scalar_tensor_tensor(pl(w1), pl(c2), 2.0 / 3.0, pl(q),
                                     ALU.mult, ALU.mult)
            vec.tensor_tensor(pl(w1), pl(w1), pl(c1), ALU.subtract)
            vec.tensor_tensor(pl(w1), pl(w1), pl(q), ALU.mult)
            vec.tensor_tensor(pl(detb), pl(w1), pl(c0), ALU.add)
            vec.tensor_tensor(pl(w1), pl(pp), pl(p2), ALU.mult)
            vec.tensor_scalar(pl(w1), pl(w1), 2.0, 1e-30, ALU.mult, ALU.add)
            vec.reciprocal(pl(w1), pl(w1))
            vec.tensor_tensor(pl(r), pl(detb), pl(w1), ALU.mult)
            vec.tensor_scalar(pl(r), pl(r), -1.0, 1.0, ALU.max, ALU.min)
            vec.tensor_tensor(pl(w1), pl(r), pl(r), ALU.mult)
            act.activation(pl(w1), pl(w1), ACT.Sqrt, scale=-1.0,
                           bias=cb(1.0 + 1e-12))                       # u
            vec.tensor_scalar_mul(pl(w2), pl(r), -1.0)
            vec.tensor_tensor(pl(w2), pl(w2), pl(r), ALU.max)          # |r|
            vec.tensor_tensor(pl(w3), pl(w2), pl(w1), ALU.min)
            vec.tensor_tensor(pl(w4), pl(w2), pl(w1), ALU.max)
            vec.reciprocal(pl(w4), pl(w4))
            vec.tensor_tensor(pl(w3), pl(w3), pl(w4), ALU.mult)
            act.activation(pl(w3), pl(w3), ACT.Arctan, bias=cb(0.0))   # phi
            vec.tensor_tensor(pl(w4), pl(w2), pl(w1), ALU.is_gt)       # g
            vec.tensor_tensor(pl(w2), pl(r), zb, ALU.is_lt)            # s
            vec.tensor_tensor(pl(w1), pl(w2), pl(w4), ALU.mult)        # s*g
            vec.tensor_scalar(pl(w1), pl(w1), np.pi, None, ALU.mult)
            vec.tensor_scalar(pl(th), pl(w4), -np.pi / 2.0, np.pi / 2.0,
                              ALU.mult, ALU.add)
            vec.tensor_tensor(pl(w1), pl(w1), pl(th), ALU.add)         # A
            vec.tensor_scalar(pl(w2), pl(w2), -2.0, 1.0, ALU.mult, ALU.add)
            vec.tensor_scalar(pl(w4), pl(w4), 2.0, -1.0, ALU.mult, ALU.add)
            vec.tensor_tensor(pl(w2), pl(w2), pl(w4), ALU.mult)        # B
            vec.tensor_tensor(pl(w3), pl(w3), pl(w2), ALU.mult)
            vec.tensor_tensor(pl(th), pl(w1), pl(w3), ALU.add)         # acos
            vec.tensor_scalar_mul(pl(w4), pl(pp), 2.0)                 # 2p
            act.activation(pl(w1), pl(th), ACT.Sin, scale=-1.0 / 3.0,
                           bias=cb(np.pi / 2.0))
            act.activation(pl(w2), pl(th), ACT.Sin, scale=1.0 / 3.0,
                           bias=cb(-np.pi / 6.0))
            act.activation(pl(w3), pl(th), ACT.Sin, scale=1.0 / 3.0,
                           bias=cb(np.pi / 6.0))
            vec.tensor_tensor(pl(w1), pl(w1), pl(w4), ALU.mult)
            vec.tensor_tensor(pl(l1), pl(w1), pl(q), ALU.add)
            vec.tensor_tensor(pl(w2), pl(w2), pl(w4), ALU.mult)
            vec.tensor_tensor(pl(l2), pl(w2), pl(q), ALU.add)
            vec.tensor_tensor(pl(w3), pl(w3), pl(w4), ALU.mult)
            vec.tensor_tensor(pl(l3), pl(q), pl(w3), ALU.subtract)

            vec.tensor_scalar_mul(pl(c2sq), pl(c2), 2.0)               # 2c2

            def polish(l, guard):
                vec.scalar_tensor_tensor(pl(w1), pl(l), -1.0, pl(c2),
                                         ALU.mult, ALU.add)            # c2-l
                vec.tensor_tensor(pl(w1), pl(w1), pl(l), ALU.mult)
                vec.tensor_tensor(pl(w1), pl(w1), pl(c1), ALU.subtract)
                vec.tensor_tensor(pl(w1), pl(w1), pl(l), ALU.mult)
                vec.tensor_tensor(pl(w1), pl(w1), pl(c0), ALU.add)     # f
                vec.scalar_tensor_tensor(pl(w2), pl(l), -3.0, pl(c2sq),
                                         ALU.mult, ALU.add)
                vec.tensor_tensor(pl(w2), pl(w2), pl(l), ALU.mult)
                vec.tensor_tensor(pl(w2), pl(w2), pl(c1), ALU.subtract)
                vec.tensor_scalar(pl(w2), pl(w2), guard, None, ALU.add)
                vec.reciprocal(pl(w2), pl(w2))
                vec.tensor_tensor(pl(w1), pl(w1), pl(w2), ALU.mult)
                vec.tensor_tensor(pl(l), pl(l), pl(w1), ALU.subtract)
            polish(l3, -1e-20)
            if POLISH2:
                polish(l2, 1e-20)
            vec.tensor_scalar(pl(l1), pl(l1), 1e-25, None, ALU.max)
            vec.tensor_scalar(pl(l2), pl(l2), 1e-25, None, ALU.max)
            vec.tensor_scalar(pl(l3), pl(l3), 1e-25, None, ALU.max)

            act.activation(pl(al1), pl(l1), ACT.Sqrt, bias=cb(0.0))
            act.activation(pl(al2), pl(l2), ACT.Sqrt, bias=cb(0.0))
            act.activation(pl(al3), pl(l3), ACT.Sqrt, bias=cb(0.0))
            vec.reciprocal(pl(a1), pl(al1))
            vec.reciprocal(pl(a2), pl(al2))
            vec.reciprocal(pl(a3), pl(al3))

            # divided differences (sigma on l3 via sneg select)
            vec.tensor_tensor(pl(w1), pl(al1), pl(al2), ALU.add)
            vec.reciprocal(pl(w1), pl(w1))
            vec.tensor_tensor(pl(w2), pl(a1), pl(a2), ALU.mult)
            vec.scalar_tensor_tensor(pl(d12), pl(w2), -1.0, pl(w1),
                                     ALU.mult, ALU.mult)
            vec.tensor_tensor(pl(w3), pl(al2), pl(al3), ALU.add)
            vec.reciprocal(pl(w4), pl(w3))
            vec.tensor_tensor(pl(w2), pl(a2), pl(a3), ALU.mult)
            vec.scalar_tensor_tensor(pl(d23), pl(w2), -1.0, pl(w4),
                                     ALU.mult, ALU.mult)               # d23p
            vec.tensor_tensor(pl(w2), pl(l2), pl(l3), ALU.subtract)
            vec.tensor_scalar(pl(w2), pl(w2), 1e-20, None, ALU.add)
            vec.reciprocal(pl(w2), pl(w2))
            vec.tensor_tensor(pl(w4), pl(a2), pl(a3), ALU.add)
            vec.tensor_tensor(pl(w4), pl(w4), pl(w2), ALU.mult)        # d23m
            vec.tensor_tensor(pl(w2), pl(w4), pl(d23), ALU.subtract)
            vec.tensor_tensor(pl(w2), pl(w2), pl(sneg), ALU.mult)
            vec.tensor_tensor(pl(d23), pl(d23), pl(w2), ALU.add)
            # w1 still = 1/(al1+al2); w3 = al2+al3
            vec.tensor_tensor(pl(w2), pl(al3), pl(al1), ALU.add)
            vec.tensor_tensor(pl(w3), pl(w3), pl(w2), ALU.mult)
            vec.reciprocal(pl(w3), pl(w3))
            vec.tensor_tensor(pl(w2), pl(a1), pl(a2), ALU.mult)
            vec.tensor_tensor(pl(w2), pl(w2), pl(a3), ALU.mult)
            vec.tensor_tensor(pl(w4), pl(al1), pl(al2), ALU.add)
            vec.tensor_tensor(pl(w4), pl(w4), pl(al3), ALU.add)        # S
            vec.tensor_tensor(pl(w4), pl(w4), pl(w1), ALU.mult)
            vec.tensor_tensor(pl(w4), pl(w4), pl(w3), ALU.mult)
            vec.tensor_tensor(pl(d123), pl(w4), pl(w2), ALU.mult)      # d123p
            vec.tensor_tensor(pl(w2), pl(l1), pl(l3), ALU.subtract)
            vec.tensor_scalar(pl(w2), pl(w2), 1e-20, None, ALU.add)
            vec.reciprocal(pl(w2), pl(w2))
            vec.tensor_tensor(pl(w4), pl(d12), pl(d23), ALU.subtract)
            vec.tensor_tensor(pl(w4), pl(w4), pl(w2), ALU.mult)        # d123m
            vec.tensor_tensor(pl(w4), pl(w4), pl(d123), ALU.subtract)
            vec.tensor_tensor(pl(w4), pl(w4), pl(sneg), ALU.mult)
            vec.tensor_tensor(pl(d123), pl(d123), pl(w4), ALU.add)

            # Phi = d123*(K-l1)(K-l2) + d12*(K-l1) + a1*I ; R = Y @ Phi
            vec.tensor_copy(flat(Bm), flat(K))
            vec.tensor_tensor(diag(K), diag(K), bc3(l1), ALU.subtract)
            vec.tensor_tensor(diag(Bm), diag(Bm), bc3(l2), ALU.subtract)
            for k in range(3):
                a = v(K, k, (E, G), (3, 3), (0, 3))
                b = v(Bm, 3 * k, (E, G), (0, 3), (1, 3))
                if k == 0:
                    vec.tensor_tensor(mat(Mt), a, b, ALU.mult)
                else:
                    vec.tensor_tensor(mat(t1), a, b, ALU.mult)
                    vec.tensor_tensor(mat(Mt), mat(Mt), mat(t1), ALU.add)
            vec.tensor_tensor(flat(Mt), flat(Mt), bc9(d123), ALU.mult)
            vec.tensor_tensor(flat(t1), flat(K), bc9(d12), ALU.mult)
            vec.tensor_tensor(flat(Mt), flat(Mt), flat(t1), ALU.add)
            vec.tensor_tensor(diag(Mt), diag(Mt), bc3(a1), ALU.add)
            for k in range(3):
                a = v(cam, t0 * E + k, (E, G), (3, 3), (0, 3))
                b = v(Mt, 3 * k, (E, G), (0, 3), (1, 3))
                if k == 0:
                    vec.tensor_tensor(mat(Rt), a, b, ALU.mult)
                else:
                    vec.tensor_tensor(mat(t1), a, b, ALU.mult)
                    vec.tensor_tensor(mat(Rt), mat(Rt), mat(t1), ALU.add)

            nc.sync.dma_start(out=v(y_flat, t0 * E, (1, NE)), in_=flat(Rt))

        # ---- emission order: chunks 0-3, group 0 tail, rest, group 1 ----
        group_after = {0: 3, 1: len(CHUNKS) - 1}   # group gi after chunk idx
        for ci, (t0, nt) in enumerate(CHUNKS):
            emit_chunk(t0, nt)
            for gi, (g0, G) in enumerate(GROUPS):
                if group_after.get(gi) == ci and STAGE > 2:
                    emit_group(gi, g0, G)
        if STAGE <= 2:
            nc.sync.dma_start(out=y_flat, in_=v(cam, 0, (1, TPC * E)))


def build(b_local=B_LOCAL):
    nc = bacc.Bacc("TRN2", target_bir_lowering=False, debug=False)
    x = nc.dram_tensor("x", [b_local, C, 3, 3], F32, kind="ExternalInput")
    w16 = nc.dram_tensor("w16", [P, CE], F16, kind="ExternalInput")
    wm9 = nc.dram_tensor("wm9", [NCH, P, E], F16, kind="ExternalInput")
    idt = nc.dram_tensor("idt", [P, P], F16, kind="ExternalInput")
    y = nc.dram_tensor("y", [b_local, 3, 3], F32, kind="ExternalOutput")
    with TileContext(nc) as tc:
        _emit(nc, tc, x.ap(), w16.ap(), wm9.ap(), idt.ap(), y.ap())
    nc.compile()
    return nc


_NC_CACHE = {}


def make_in_maps(x: np.ndarray, W: np.ndarray):
    xs = np.ascontiguousarray(x.reshape(N_CORES, B_LOCAL, C, 3, 3))
    W = np.asarray(W, dtype=np.float32)
    w16 = make_w16(W)
    wm9 = make_wm9(W)
    idt = make_idt()
    return [{"x": xs[i], "w16": w16, "wm9": wm9, "idt": idt}
            for i in range(N_CORES)]


def kernel(x: np.ndarray, W: np.ndarray) -> np.ndarray:
    assert x.shape == (B_FULL, C, 3, 3) and W.shape == (C,)
    if "nc" not in _NC_CACHE:
        _NC_CACHE["nc"] = build()
    nc = _NC_CACHE["nc"]
    in_maps = make_in_maps(x, W)
    res = bass_utils.run_bass_kernel_spmd(nc, in_maps, core_ids=list(range(N_CORES)))
    return np.concatenate([r["y"] for r in res.results], axis=0)


if __name__ == "__main__":
    rng = np.random.default_rng(0)
    x = rng.standard_normal((B_FULL, C, 3, 3), dtype=np.float32)
    W = (rng.standard_normal(C, dtype=np.float32) / np.sqrt(C)).astype(np.float32)
    out = kernel(x=x, W=W)
    print(out.shape, out.dtype)
